# revision 29
# baseline (speedup 1.0000x reference)
"""Trainium2 Bass kernel for nn_CoordinateLinear: out = W @ x + b.

Full shapes: x [1024, 16384] f32, W [1024, 1024] f32, b [1024] f32,
out [1024, 16384] f32.

Sharding: data-parallel on the batch axis — each of the 8 cores computes
out[:, c*2048:(c+1)*2048] = W @ x[:, c*2048:(c+1)*2048] + b with W and b
replicated.

Per-core device kernel: K=1024 contraction in 8 tiles of 128 partitions,
M=1024 output rows in 8 tiles of 128, N=2048 batch in 4 slabs of 512.
W^T is resident in SBUF; x streams per n-slab (double buffered); PSUM
accumulates over k; the PSUM->SBUF eviction fuses the bias add on the
scalar engine.

MODE selects the matmul numerics/throughput tradeoff:
  "f32"    exact fp32 matmul, 4 PE cycles/row.
  "f32r"   single-pass fp32r (fp32 rounded to 11-bit mantissa on host),
           1 PE cycle/row — 4x faster, ~1e-4 relative error.
  "split3" bf16 hi/lo split, out = Wh@xh + Wh@xl + Wl@xh, 3 passes at
           1 cycle/row — 1.33x faster than f32, fp32-quality error.
"""

import sys

if "/opt/trn_rl_repo" not in sys.path:
    sys.path.insert(0, "/opt/trn_rl_repo")

import ml_dtypes
import numpy as np

import concourse.bass as bass
import concourse.mybir as mybir
import concourse.tile as tile
from concourse import bacc
from concourse.bass_utils import run_bass_kernel_spmd

N_CORES = 8
P = 128
K = 1024
M = 1024
N_FULL = 16384
N_CORE = N_FULL // N_CORES  # 2048
N_TILE = 512
K_T = K // P  # 8
M_T = M // P  # 8
N_T = N_CORE // N_TILE  # 4

MODE = "f32r"

BF16 = ml_dtypes.bfloat16
_compiled = {}


def _round_fp32r(a):
    """Round fp32 to fp32r (1s + 8e + 11m in the top 20 bits), RNE."""
    u = np.ascontiguousarray(a, dtype=np.float32).view(np.uint32)
    r = (u + np.uint32(0x7FF) + ((u >> np.uint32(12)) & np.uint32(1))) & np.uint32(
        0xFFFFF000
    )
    return r.view(np.float32)


def _mode_cfg(mode):
    f32 = mybir.dt.float32
    if mode == "f32":
        return f32, ["wT"], ["x"], [("wT", "x")]
    if mode == "f32r":
        return mybir.dt.float32r, ["wT"], ["x"], [("wT", "x")]
    if mode == "split3":
        return mybir.dt.bfloat16, ["wT_hi", "wT_lo"], ["x_hi", "x_lo"], [
            ("wT_hi", "x_hi"),
            ("wT_hi", "x_lo"),
            ("wT_lo", "x_hi"),
        ]
    raise ValueError(mode)


def _build(mode, repeat=1, bench_internal=False, evict="act", probe=None):
    # probe (bench-only): "noout" drops the output DMAs; "nox" drops the x
    # slab loads (matmuls read stale slabs); isolates the steady-state limiter.
    nc = bacc.Bacc("TRN2", target_bir_lowering=False, debug=False)

    f32 = mybir.dt.float32
    in_dt, w_names, x_names, terms = _mode_cfg(mode)

    io_kind = "Internal" if bench_internal else None
    w_kind = io_kind or "ExternalInput"
    w_d = {nm: nc.dram_tensor(nm, [K, M], in_dt, kind=w_kind) for nm in w_names}
    x_d = {nm: nc.dram_tensor(nm, [K, N_CORE], in_dt, kind=w_kind) for nm in x_names}
    b_d = nc.dram_tensor("bias", [P, M_T], f32, kind=w_kind)
    o_d = nc.dram_tensor("out", [M, N_CORE], f32, kind=io_kind or "ExternalOutput")
    if bench_internal:
        tok_i = nc.dram_tensor("tok_i", [P, 16], f32, kind="ExternalInput")
        tok_o = nc.dram_tensor("tok_o", [P, 16], f32, kind="ExternalOutput")

    x_bufs = 8 if probe == "wreuse" else 3
    with tile.TileContext(nc) as tc:
        with (
            tc.tile_pool(name="wpool", bufs=1) as wpool,
            tc.tile_pool(name="xpool", bufs=x_bufs) as xpool,
            tc.tile_pool(name="bpool", bufs=1) as bpool,
            tc.tile_pool(name="opool", bufs=6) as opool,
            tc.tile_pool(name="pspool", bufs=8, space="PSUM") as pspool,
        ):
            if bench_internal:
                tok_sb = bpool.tile([P, 16], f32, tag="tok")
                nc.sync.dma_start(out=tok_sb[:], in_=tok_i[:])
                nc.sync.dma_start(out=tok_o[:], in_=tok_sb[:])

            # repeat == 0 builds a "null" benchmark NEFF: token roundtrip only.
            bias_sb = None
            w_sb = {}
            if repeat > 0:
                bias_sb = bpool.tile([P, M_T], f32, tag="bias")
                nc.sync.dma_start(out=bias_sb[:], in_=b_d[:])

                # Resident weights: w_sb[nm] is [P, K_T * M]; k-tile k lives
                # at free-dim offset k*M. Loaded per (m, k) tile, m-major, so
                # the first m-groups' weights land before the whole matrix
                # arrives.
                for nm in w_names:
                    t = wpool.tile([P, K_T * M], in_dt, tag=f"w_{nm}", name=f"{nm}_sb")
                    for m in range(M_T):
                        for k in range(K_T):
                            nc.sync.dma_start(
                                out=t[:, k * M + m * P : k * M + (m + 1) * P],
                                in_=w_d[nm][k * P : (k + 1) * P, m * P : (m + 1) * P],
                            )
                    w_sb[nm] = t

            n_mm = len(terms) * K_T
            if probe == "wreuse":
                _emit_wreuse(
                    nc, xpool, pspool, opool, x_d, x_names, w_sb, bias_sb, o_d,
                    in_dt, terms, repeat, evict,
                )
                repeat = 0  # skip the standard loop below
            for r in range(repeat):
                for n in range(N_T):
                    # Stream this n-slab of x: [P, K_T * N_TILE] per tensor,
                    # k-tile k at free-dim offset k*N_TILE.
                    x_sb = {}
                    for nm in x_names:
                        t = xpool.tile(
                            [P, K_T * N_TILE],
                            in_dt,
                            tag=f"x_{nm}",
                            name=f"{nm}_slab_{r}_{n}",
                        )
                        k_load = 1 if probe == "nox" else K_T
                        for k in range(k_load):
                            nc.sync.dma_start(
                                out=t[:, k * N_TILE : (k + 1) * N_TILE],
                                in_=x_d[nm][
                                    k * P : (k + 1) * P,
                                    n * N_TILE : (n + 1) * N_TILE,
                                ],
                            )
                        x_sb[nm] = t

                    for m in range(M_T):
                        ps = pspool.tile(
                            [P, N_TILE], f32, tag="ps", name=f"ps_{r}_{n}_{m}"
                        )
                        i = 0
                        for wn, xn in terms:
                            for k in range(K_T):
                                nc.tensor.matmul(
                                    ps[:],
                                    w_sb[wn][:, k * M + m * P : k * M + (m + 1) * P],
                                    x_sb[xn][:, k * N_TILE : (k + 1) * N_TILE],
                                    start=(i == 0),
                                    stop=(i == n_mm - 1),
                                )
                                i += 1
                        o_sb = opool.tile(
                            [P, N_TILE], f32, tag="o", name=f"o_{r}_{n}_{m}"
                        )
                        if evict == "act":
                            nc.scalar.activation(
                                o_sb[:],
                                ps[:],
                                mybir.ActivationFunctionType.Identity,
                                bias=bias_sb[:, m : m + 1],
                            )
                        else:
                            nc.vector.tensor_scalar_add(
                                o_sb[:], ps[:], bias_sb[:, m : m + 1]
                            )
                        if probe != "noout":
                            nc.sync.dma_start(
                                out=o_d[
                                    m * P : (m + 1) * P, n * N_TILE : (n + 1) * N_TILE
                                ],
                                in_=o_sb[:],
                            )

    nc.compile()
    return nc


def _emit_wreuse(
    nc, xpool, pspool, opool, x_d, x_names, w_sb, bias_sb, o_d, in_dt, terms,
    repeat, evict,
):
    """Loop order m -> k -> n: 4 PSUM banks accumulate in parallel and 4
    consecutive matmuls share the same stationary weight tile."""
    f32 = mybir.dt.float32
    for r in range(repeat):
        x_sb = {}
        for n in range(N_T):
            for nm in x_names:
                x_sb[(nm, n)] = xpool.tile(
                    [P, K_T * N_TILE], in_dt, tag=f"x_{nm}", name=f"{nm}_sl_{r}_{n}"
                )
        # k-major emission: the first m-group's k0 slices arrive first.
        for k in range(K_T):
            for n in range(N_T):
                for nm in x_names:
                    nc.sync.dma_start(
                        out=x_sb[(nm, n)][:, k * N_TILE : (k + 1) * N_TILE],
                        in_=x_d[nm][
                            k * P : (k + 1) * P, n * N_TILE : (n + 1) * N_TILE
                        ],
                    )
        n_mm = len(terms) * K_T
        for m in range(M_T):
            pss = [
                pspool.tile([P, N_TILE], f32, tag="ps", name=f"ps_{r}_{m}_{n}")
                for n in range(N_T)
            ]
            i = 0
            for wn, xn in terms:
                for k in range(K_T):
                    for n in range(N_T):
                        nc.tensor.matmul(
                            pss[n][:],
                            w_sb[wn][:, k * M + m * P : k * M + (m + 1) * P],
                            x_sb[(xn, n)][:, k * N_TILE : (k + 1) * N_TILE],
                            start=(i == 0),
                            stop=(i == n_mm - 1),
                        )
                    i += 1
            for n in range(N_T):
                o_sb = opool.tile([P, N_TILE], f32, tag="o", name=f"o_{r}_{m}_{n}")
                nc.scalar.activation(
                    o_sb[:],
                    pss[n][:],
                    mybir.ActivationFunctionType.Identity,
                    bias=bias_sb[:, m : m + 1],
                )
                nc.sync.dma_start(
                    out=o_d[m * P : (m + 1) * P, n * N_TILE : (n + 1) * N_TILE],
                    in_=o_sb[:],
                )


def _get_compiled(mode, repeat=1, bench_internal=False, evict="act", probe=None):
    key = (mode, repeat, bench_internal, evict, probe)
    if key not in _compiled:
        _compiled[key] = _build(mode, repeat, bench_internal, evict, probe)
    return _compiled[key]


def _in_maps(mode, x, weight, bias):
    x = np.asarray(x, dtype=np.float32)
    wT = np.ascontiguousarray(np.asarray(weight, dtype=np.float32).T)
    b_pre = np.ascontiguousarray(np.asarray(bias, dtype=np.float32).reshape(M_T, P).T)

    if mode == "f32":
        w_parts = {"wT": wT}
        x_full = {"x": x}
    elif mode == "f32r":
        w_parts = {"wT": _round_fp32r(wT)}
        x_full = {"x": _round_fp32r(x)}
    elif mode == "split3":
        wh = wT.astype(BF16)
        wl = (wT - wh.astype(np.float32)).astype(BF16)
        xh = x.astype(BF16)
        xl = (x - xh.astype(np.float32)).astype(BF16)
        w_parts = {"wT_hi": wh, "wT_lo": wl}
        x_full = {"x_hi": xh, "x_lo": xl}
    else:
        raise ValueError(mode)

    maps = []
    for c in range(N_CORES):
        m = dict(w_parts)
        m["bias"] = b_pre
        for nm, arr in x_full.items():
            m[nm] = np.ascontiguousarray(arr[:, c * N_CORE : (c + 1) * N_CORE])
        maps.append(m)
    return maps


LOOP = "wreuse"  # "wreuse" (m->k->n, 4 PSUM banks) or None (n->m->k)


def kernel(x, weight, bias):
    nc = _get_compiled(MODE, probe=LOOP)
    maps = _in_maps(MODE, x, weight, bias)
    last_err = None
    for _ in range(3):
        try:
            res = run_bass_kernel_spmd(nc, maps, core_ids=list(range(N_CORES)))
            break
        except Exception as exc:  # transient NRT device errors; retry
            last_err = exc
    else:
        raise last_err
    out = np.concatenate([res.results[c]["out"] for c in range(N_CORES)], axis=1)
    return np.ascontiguousarray(out, dtype=np.float32)
